# revision 16
# baseline (speedup 1.0000x reference)
"""Trainium2 Bass kernel for nn_AttentionBlock (sparse sliding-window attention).

Sharding: pure data-parallel over tokens. B=4 batches x 2 T-halves of 512
tokens = 8 shards, one per NeuronCore. Sliding-window(128) causal attention
only needs a 128-token K/V halo per shard, so there is no cross-core
communication at all. Each core runs: QKV projection -> RoPE -> windowed
attention (with per-head sink folded into the softmax denominator) -> output
projection for its 512 tokens.

On-chip layouts (per core):
  xT        [dmodel, 640tok]   (halo 128 + 512 own tokens; halo zero-padded
                                for the first half of each sequence)
  Q.T, K.T  [64feat, tok]      per-head projection output; natural operands
                               for the transposed-scores matmul S.T = K^T Q
  V         [tok, feat]        built by PE-transposing the V.T projection;
                               augmented with a ones column per head so the
                               softmax denominator falls out of the PV matmul
  scores    S.T [key, query]   per (head, 256-query-block, 128-key-block)
  softmax   no max-subtraction (logits are bounded); multiplicative 0/1 mask
            applied after exp; normalization of O.T via gpsimd
            partition_broadcast of 1/denom + elementwise multiply
  out proj  Y [tok, dmodel] accumulated in PSUM over 16 feature tiles with
            the bias folded in as a K=1 rank-1 matmul
All matmuls run as float32r (fp32 storage, full-rate PE mode): weight/x
operands are DMA'd as f32r, compute-produced operands get their f32r
rounding from the last DVE op that writes them.
"""

import math
from contextlib import ExitStack

import numpy as np

import concourse.bacc as bacc
import concourse.tile as tile
from concourse import mybir
from concourse.bass_utils import run_bass_kernel_spmd

_DEBUG = False
F32 = mybir.dt.float32
F32R = mybir.dt.float32r
AF = mybir.ActivationFunctionType
ALU = mybir.AluOpType

B, T, D = 4, 1024, 2048
HEAD_DIM = 64
N_HEADS = 32
N_KV = 8
WINDOW = 128
SM_SCALE = 1.0 / math.sqrt(HEAD_DIM)
ROPE_THETA = 150000.0
SCALING = 32.0
NTK_ALPHA = 1.0
NTK_BETA = 32.0
ICL = 1024

TQ = 512          # queries per shard
HALO = 128
TOK = TQ + HALO   # 640 tokens of K/V context per shard
NKT = D // 128    # 16 contraction tiles over dmodel
NQF = 16          # Q feature tiles (2048 features)
NKF = 4           # K feature tiles (512)
NVF = 4           # V feature tiles (512)
NTT = TOK // 128  # 5 token tiles


def _build_nc():
    nc = bacc.Bacc("TRN2", target_bir_lowering=False, debug=False)

    # ---- DRAM I/O ----
    xt = nc.dram_tensor("xt", (NKT, 128, TOK), F32R, kind="ExternalInput")
    wqk = nc.dram_tensor("wqk", (24, 128, D), F32R, kind="ExternalInput")
    wo = nc.dram_tensor("wo", (16, 128, D), F32R, kind="ExternalInput")
    qkvb = nc.dram_tensor("qkvb", (128, 24), F32, kind="ExternalInput")
    qkvb2 = nc.dram_tensor("qkvb2", (64, 48), F32, kind="ExternalInput")
    outb = nc.dram_tensor("outb", (1, D), F32R, kind="ExternalInput")
    esink = nc.dram_tensor("esink", (1, N_HEADS), F32, kind="ExternalInput")
    costab = nc.dram_tensor("costab", (64, TOK), F32, kind="ExternalInput")
    sintab = nc.dram_tensor("sintab", (64, TOK), F32, kind="ExternalInput")
    mask = nc.dram_tensor("mask", (128, 6, 512), mybir.dt.bfloat16, kind="ExternalInput")
    ones1 = nc.dram_tensor("ones1", (1, 128), F32R, kind="ExternalInput")
    pairsel = nc.dram_tensor("pairsel", (128, 2, 128), F32R, kind="ExternalInput")
    ident = nc.dram_tensor("ident", (128, 128), F32, kind="ExternalInput")
    vones = nc.dram_tensor("vones", (128, N_KV, 1), F32R, kind="ExternalInput")
    y = nc.dram_tensor("y", (TQ, D), F32, kind="ExternalOutput")
    if _DEBUG:
        dbg_q = nc.dram_tensor("dbg_q", (64, TQ), F32R, kind="ExternalOutput")
        dbg_k = nc.dram_tensor("dbg_k", (64, TOK), F32R, kind="ExternalOutput")
        dbg_hd = nc.dram_tensor("dbg_hd", (64, TOK), F32, kind="ExternalOutput")
        dbg_v = nc.dram_tensor("dbg_v", (128, 65), F32R, kind="ExternalOutput")
        dbg_pt = nc.dram_tensor("dbg_pt", (128, 256), F32R, kind="ExternalOutput")
        dbg_opk = nc.dram_tensor("dbg_opk", (128, 256), F32, kind="ExternalOutput")
        dbg_rpk = nc.dram_tensor("dbg_rpk", (2, 256), F32, kind="ExternalOutput")
        dbg_rt = nc.dram_tensor("dbg_rt", (128, 256), F32, kind="ExternalOutput")
        dbg_opkn = nc.dram_tensor("dbg_opkn", (128, 256), F32R, kind="ExternalOutput")

    with tile.TileContext(nc) as tc, ExitStack() as ctx:
        ep = ctx.enter_context
        const = ep(tc.tile_pool(name="const", bufs=1))
        hdp = ep(tc.tile_pool(name="hdp", bufs=3))       # pre-rope per-head F32
        swp = ep(tc.tile_pool(name="swp", bufs=2))       # rope swap + product
        ktp = ep(tc.tile_pool(name="ktp", bufs=2 * NKF))  # rotated K (f32r)
        vtp = ep(tc.tile_pool(name="vtp", bufs=2))
        vp = ep(tc.tile_pool(name="vp", bufs=NTT))
        qtp = ep(tc.tile_pool(name="qtp", bufs=4))       # rotated Q (f32r)
        wqkp = ep(tc.tile_pool(name="wqkp", bufs=2))
        opackp = ep(tc.tile_pool(name="opackp", bufs=32))
        opkrawp = ep(tc.tile_pool(name="opkraw", bufs=3))
        rtlp = ep(tc.tile_pool(name="rtlp", bufs=2))
        ptp = ep(tc.tile_pool(name="ptp", bufs=3))
        yp = ep(tc.tile_pool(name="yp", bufs=2))
        rpkp = ep(tc.tile_pool(name="rpkp", bufs=2))
        xctx = ExitStack()
        xtp = xctx.enter_context(tc.tile_pool(name="xtp", bufs=NKT))
        if True:
            # ---- constants ----
            qkvb_sb = const.tile([128, 24], F32)
            nc.sync.dma_start(out=qkvb_sb, in_=qkvb[:])
            qkvb2_sb = const.tile([64, 48], F32)
            nc.sync.dma_start(out=qkvb2_sb, in_=qkvb2[:])
            outb_sb = const.tile([1, D], F32R)
            nc.sync.dma_start(out=outb_sb, in_=outb[:])
            esink_sb = const.tile([1, N_HEADS], F32)
            nc.sync.dma_start(out=esink_sb, in_=esink[:])
            cos_sb = const.tile([64, TOK], F32)
            nc.sync.dma_start(out=cos_sb, in_=costab[:])
            sin_sb = const.tile([64, TOK], F32)
            nc.sync.dma_start(out=sin_sb, in_=sintab[:])
            mask_sb = const.tile([128, 6, 512], mybir.dt.bfloat16)
            nc.sync.dma_start(out=mask_sb, in_=mask[:])
            ones_sb = const.tile([1, 128], F32R)
            nc.sync.dma_start(out=ones_sb, in_=ones1[:])
            pair_sb = const.tile([128, 2, 128], F32R)
            nc.sync.dma_start(out=pair_sb, in_=pairsel[:])
            ident_sb = const.tile([128, 128], F32)
            nc.sync.dma_start(out=ident_sb, in_=ident[:])


            # ---- x.T tiles ----
            wsb_first = wqkp.tile([128, NKT, 128], F32R, tag="wqk")
            nc.sync.dma_start(out=wsb_first, in_=wqk[16])
            xts = []
            for kt in range(NKT):
                t = xtp.tile([128, TOK], F32R, tag="xt")
                nc.sync.dma_start(out=t, in_=xt[kt])
                xts.append(t)

            def rope(hd, dst, tok0, ntok):
                """dst (f32r) = rotary(hd); hd is a [64, ntok] f32 per-head
                tile (rows 0-31 first halves, 32-63 second halves)."""
                swap_t = swp.tile([64, TOK], F32, tag="swap")
                nc.gpsimd.dma_start(out=swap_t[0:32, :ntok], in_=hd[32:64, :])
                nc.gpsimd.dma_start(out=swap_t[32:64, :ntok], in_=hd[0:32, :])
                prod_t = swp.tile([64, TOK], F32, tag="prod")
                nc.gpsimd.tensor_mul(
                    out=prod_t[:, :ntok], in0=hd, in1=cos_sb[:, tok0 : tok0 + ntok]
                )
                nc.gpsimd.tensor_mul(
                    out=swap_t[:, :ntok],
                    in0=swap_t[:, :ntok],
                    in1=sin_sb[:, tok0 : tok0 + ntok],
                )
                nc.vector.tensor_add(
                    out=dst, in0=prod_t[:, :ntok], in1=swap_t[:, :ntok]
                )

            def evac_heads(pst, c0, nt, dst0, dst1, f):
                """Evacuate a [128, nt] projection PSUM chunk into two
                per-head [64, *] F32 SBUF tiles, adding the bias."""
                nc.scalar.activation(
                    out=dst0[:, c0 : c0 + nt],
                    in_=pst[0:64, :],
                    func=AF.Identity,
                    bias=qkvb2_sb[:, 2 * f : 2 * f + 1],
                )
                nc.vector.tensor_scalar(
                    out=dst1[:, c0 : c0 + nt],
                    in0=pst[64:128, :],
                    scalar1=qkvb2_sb[:, 2 * f + 1 : 2 * f + 2],
                    scalar2=None,
                    op0=ALU.add,
                )

            pctx = ExitStack()
            ps512 = pctx.enter_context(tc.tile_pool(name="ps512", bufs=4, space="PSUM"))
            ps256 = pctx.enter_context(tc.tile_pool(name="ps256", bufs=2, space="PSUM"))
            ps65 = pctx.enter_context(tc.tile_pool(name="ps65", bufs=2, space="PSUM"))
            if True:
                # ---- K.T projection (feature tiles 16..19) + RoPE ----
                kth = [None] * N_KV
                for fk in range(NKF):
                    f = 16 + fk
                    if fk == 0:
                        wsb = wsb_first
                    else:
                        wsb = wqkp.tile([128, NKT, 128], F32R, tag="wqk")
                        nc.sync.dma_start(out=wsb, in_=wqk[f])
                    k0 = hdp.tile([64, TOK], F32, tag="hd")
                    k1 = hdp.tile([64, TOK], F32, tag="hd")
                    for c0 in (0, 320):
                        pst_full = ps512.tile([128, 512], F32, tag="ps512")
                        pst = pst_full[:, 0:320]
                        for k in range(NKT):
                            nc.tensor.matmul(
                                pst,
                                wsb[:, k, :],
                                xts[k][:, c0 : c0 + 320],
                                start=(k == 0),
                                stop=(k == NKT - 1),
                            )
                        evac_heads(pst, c0, 320, k0, k1, f)
                    kr0 = ktp.tile([64, TOK], F32R, tag="kt")
                    kr1 = ktp.tile([64, TOK], F32R, tag="kt")
                    rope(k0, kr0, 0, TOK)
                    rope(k1, kr1, 0, TOK)
                    kth[2 * fk] = kr0
                    kth[2 * fk + 1] = kr1
                    if _DEBUG and fk == 0:
                        nc.sync.dma_start(out=dbg_k[:], in_=kr0)
                        nc.sync.dma_start(out=dbg_hd[:], in_=k0)

                # ---- V.T projection (feature tiles 20..23) -> transpose to V ----
                vsbs = []
                for tt in range(NTT):
                    vsb = vp.tile([128, N_KV, 65], F32R, tag="v")
                    nc.gpsimd.dma_start(out=vsb[:, :, 64:65], in_=vones[:])
                    vsbs.append(vsb)
                for fv in range(NVF):
                    f = 20 + fv
                    wsb = wqkp.tile([128, NKT, 128], F32R, tag="wqk")
                    nc.sync.dma_start(out=wsb, in_=wqk[f])
                    vt_sb = vtp.tile([128, TOK], F32, tag="vt")
                    for c0 in (0, 320):
                        pst_full = ps512.tile([128, 512], F32, tag="ps512")
                        pst = pst_full[:, 0:320]
                        for k in range(NKT):
                            nc.tensor.matmul(
                                pst,
                                wsb[:, k, :],
                                xts[k][:, c0 : c0 + 320],
                                start=(k == 0),
                                stop=(k == NKT - 1),
                            )
                        nc.scalar.activation(
                            out=vt_sb[:, c0 : c0 + 320],
                            in_=pst,
                            func=AF.Identity,
                            bias=qkvb_sb[:, f : f + 1],
                        )
                    # transpose each [128,128] block: [2 heads x 64, tok] -> [tok, 2x64]
                    for tt in range(NTT):
                        tps_full = ps256.tile([128, 256], F32, tag="ps256")
                        tps = tps_full[:, 0:128]
                        nc.tensor.transpose(
                            tps, vt_sb[:, tt * 128 : (tt + 1) * 128], ident_sb
                        )
                        nc.vector.tensor_copy(
                            out=vsbs[tt][:, 2 * fv : 2 * fv + 2, 0:64],
                            in_=tps.rearrange("p (h d) -> p h d", h=2),
                        )
                if _DEBUG:
                    nc.sync.dma_start(out=dbg_v[:], in_=vsbs[2][:, 0, :])

                # ---- Q projection + attention, per feature tile (= head pair) ----
                opacks = [[None, None] for _ in range(16)]
                for qf in range(NQF):
                    wsb = wqkp.tile([128, NKT, 128], F32R, tag="wqk")
                    nc.sync.dma_start(out=wsb, in_=wqk[qf])
                    q0 = hdp.tile([64, TQ], F32, tag="hd")
                    q1 = hdp.tile([64, TQ], F32, tag="hd")
                    pst = ps512.tile([128, 512], F32, tag="ps512")
                    for k in range(NKT):
                        nc.tensor.matmul(
                            pst,
                            wsb[:, k, :],
                            xts[k][:, HALO:TOK],
                            start=(k == 0),
                            stop=(k == NKT - 1),
                        )
                    evac_heads(pst, 0, TQ, q0, q1, qf)
                    qrb = qtp.tile([64, 2, TQ], F32R, tag="qt")
                    rope(q0, qrb[:, 0, :], HALO, TQ)
                    rope(q1, qrb[:, 1, :], HALO, TQ)
                    if _DEBUG and qf == 0:
                        nc.sync.dma_start(out=dbg_q[:], in_=qrb[:, 0, :])

                    h = qf // 2  # kv head shared by both q heads in this tile
                    rpk = rpkp.tile([128, 256], F32, tag="rpk")
                    nc.vector.memset(rpk, 1.0)
                    opks = []
                    for qb in range(2):
                        opk = opkrawp.tile([128, 256], F32, tag="opkraw")
                        opks.append(opk)
                        opsum = ps65.tile([65, 512], F32, tag="ps65")
                        for kb in range(3):
                            kcol = qb * 256 + kb * 128
                            stp = ps512.tile([128, 512], F32, tag="ps512")
                            nc.tensor.matmul(
                                stp,
                                kth[h][:, kcol : kcol + 128],
                                qrb[:, :, qb * 256 : qb * 256 + 256],
                                start=True,
                                stop=True,
                            )
                            pt = ptp.tile([128, 512], F32, tag="pt")
                            nc.scalar.activation(
                                out=pt, in_=stp, func=AF.Exp, scale=SM_SCALE
                            )
                            ptr = ptp.tile([128, 512], F32R, tag="ptr")
                            nc.vector.tensor_mul(
                                out=ptr, in0=pt, in1=mask_sb[:, qb * 3 + kb, :]
                            )
                            nc.tensor.matmul(
                                opsum,
                                vsbs[qb * 2 + kb][:, h, :],
                                ptr,
                                start=(kb == 0),
                                stop=(kb == 2),
                            )
                            if _DEBUG and qf == 0 and qb == 0 and kb == 1:
                                nc.sync.dma_start(out=dbg_pt[:], in_=ptr[:, 0:256])
                        for m01 in range(2):
                            qh = 2 * qf + m01
                            rrow = 64 * qb + 32 * m01
                            nc.vector.tensor_scalar(
                                out=rpk[rrow : rrow + 1, :],
                                in0=opsum[64:65, m01 * 256 : m01 * 256 + 256],
                                scalar1=esink_sb[0:1, qh : qh + 1],
                                scalar2=None,
                                op0=ALU.add,
                            )
                        # evacuate unnormalized O.T rows
                        nc.scalar.activation(
                            out=opk[0:64, :], in_=opsum[0:64, 0:256], func=AF.Copy
                        )
                        nc.vector.tensor_copy(
                            out=opk[64:128, :], in_=opsum[0:64, 256:512]
                        )
                    # one reciprocal for all 4 denominator rows of this qf
                    nc.vector.reciprocal(out=rpk, in_=rpk)
                    rpk_r = rtlp.tile([128, 256], F32R, tag="rtile")
                    nc.vector.tensor_copy(out=rpk_r, in_=rpk)
                    for qb in range(2):
                        rps = ps256.tile([128, 256], F32, tag="ps256")
                        nc.tensor.matmul(rps, pair_sb[:, qb, :], rpk_r, start=True, stop=True)
                        opk_n = opackp.tile([128, 256], F32R, tag="opack")
                        nc.vector.tensor_mul(out=opk_n, in0=opks[qb], in1=rps)
                        opacks[qf][qb] = opk_n
                        if _DEBUG and qf == 0 and qb == 0:
                            nc.sync.dma_start(out=dbg_opk[:], in_=opks[0])
                            nc.sync.dma_start(out=dbg_rpk[0:1], in_=rpk[0:1, :])
                            nc.sync.dma_start(out=dbg_rpk[1:2], in_=rpk[32:33, :])
                            nc.sync.dma_start(out=dbg_opkn[:], in_=opk_n)

            # ---- output projection ----
            pctx.close()
            xctx.close()
            wop = ctx.enter_context(tc.tile_pool(name="wop", bufs=6))
            psy = ctx.enter_context(tc.tile_pool(name="psy", bufs=8, space="PSUM"))
            if True:
                for chp in range(2):  # two column-half passes over out_w
                    wos = []
                    for ft in range(16):
                        wosb = wop.tile([128, 1024], F32R, tag="wo")
                        nc.sync.dma_start(
                            out=wosb, in_=wo[ft][:, chp * 1024 : (chp + 1) * 1024]
                        )
                        wos.append(wosb)
                    for tqt in range(4):
                        qb, col = tqt // 2, tqt % 2
                        for c2 in range(2):
                            ch = chp * 2 + c2
                            yps = psy.tile([128, 512], F32, tag="psy")
                            for ft in range(16):
                                nc.tensor.matmul(
                                    yps,
                                    opacks[ft][qb][:, col * 128 : col * 128 + 128],
                                    wos[ft][:, c2 * 512 : c2 * 512 + 512],
                                    start=(ft == 0),
                                    stop=False,
                                )
                            nc.tensor.matmul(
                                yps,
                                ones_sb,
                                outb_sb[:, ch * 512 : ch * 512 + 512],
                                start=False,
                                stop=True,
                            )
                            ysb = yp.tile([128, 512], F32, tag="y")
                            nc.scalar.activation(out=ysb, in_=yps, func=AF.Copy)
                            nc.sync.dma_start(
                                out=y[tqt * 128 : (tqt + 1) * 128, ch * 512 : ch * 512 + 512],
                                in_=ysb,
                            )

    nc.compile()
    return nc


_NC_CACHE = None


def _get_nc():
    global _NC_CACHE
    if _NC_CACHE is None:
        _NC_CACHE = _build_nc()
    return _NC_CACHE


def _rope_tables(positions):
    """fp32 YaRN/NTK-by-parts tables, matching the reference bit-for-bit."""
    d_half = HEAD_DIM // 2
    freq = ROPE_THETA ** (np.arange(0, HEAD_DIM, 2, dtype=np.float32) / HEAD_DIM)
    concentration = 0.1 * math.log(SCALING) + 1.0
    low = d_half * math.log(ICL / (NTK_BETA * 2 * math.pi)) / math.log(ROPE_THETA)
    high = d_half * math.log(ICL / (NTK_ALPHA * 2 * math.pi)) / math.log(ROPE_THETA)
    interpolation = 1.0 / (SCALING * freq)
    extrapolation = 1.0 / freq
    ramp = np.clip(
        (np.arange(d_half, dtype=np.float32) - low) / (high - low), 0.0, 1.0
    )
    inv_freq = interpolation * ramp + extrapolation * (1.0 - ramp)
    freqs = np.outer(positions.astype(np.float32), inv_freq)  # (n, 32)
    return (
        (np.cos(freqs) * concentration).astype(np.float32),
        (np.sin(freqs) * concentration).astype(np.float32),
    )


def _host_inputs(x, qkv_w, qkv_b, out_w, out_b, sinks):
    x = np.asarray(x, np.float32)
    qkv_w = np.asarray(qkv_w, np.float32)
    qkv_b = np.asarray(qkv_b, np.float32)
    out_w = np.asarray(out_w, np.float32)
    out_b = np.asarray(out_b, np.float32)
    sinks = np.asarray(sinks, np.float32)

    wqk_h = np.ascontiguousarray(
        qkv_w.reshape(24, 128, NKT, 128).transpose(0, 3, 2, 1).reshape(24, 128, D)
    )
    wo_h = np.ascontiguousarray(out_w.T.reshape(16, 128, D))
    qkvb_h = np.ascontiguousarray(qkv_b.reshape(24, 128).T)
    qkvb2_h = np.ascontiguousarray(qkv_b.reshape(48, 64).T)
    outb_h = out_b.reshape(1, D).copy()
    esink_h = np.exp(sinks).reshape(1, N_HEADS).astype(np.float32)
    ones_h = np.ones((1, 128), np.float32)
    ident_h = np.eye(128, dtype=np.float32)
    vones_h = np.ones((128, N_KV, 1), np.float32)
    pair_h = np.zeros((128, 2, 128), np.float32)
    pair_h[0, 0, 0:64] = 1.0
    pair_h[32, 0, 64:128] = 1.0
    pair_h[64, 1, 0:64] = 1.0
    pair_h[96, 1, 64:128] = 1.0

    # masks / rope tables per T-half
    masks, tabs = [], []
    for half in range(2):
        t0 = half * TQ
        p = np.arange(128)[:, None]
        r = np.arange(256)[None, :]
        m = np.zeros((128, 6, 256), np.float32)
        for qb in range(2):
            for kb in range(3):
                dd = kb * 128 + p - r
                vis = (dd >= 1) & (dd <= 128)
                if half == 0:
                    vis = vis & ((qb * 256 + kb * 128 + p) >= HALO)
                m[:, qb * 3 + kb, :] = vis.astype(np.float32)
        import ml_dtypes
        m2 = np.concatenate([m, m], axis=2).astype(ml_dtypes.bfloat16)
        masks.append(m2)
        pos = np.clip(np.arange(t0 - HALO, t0 + TQ), 0, None)
        cos_t, sin_t = _rope_tables(pos)  # (TOK, 32)
        cos2 = np.concatenate([cos_t.T, cos_t.T], axis=0)       # (64, TOK)
        sin2 = np.concatenate([-sin_t.T, sin_t.T], axis=0)      # (64, TOK) signed
        tabs.append((np.ascontiguousarray(cos2), np.ascontiguousarray(sin2)))

    in_maps = []
    for core in range(8):
        b, half = core // 2, core % 2
        t0 = half * TQ
        x_pad = np.zeros((TOK, D), np.float32)
        lo = t0 - HALO
        x_pad[max(0, -lo) :] = x[b, max(lo, 0) : t0 + TQ]
        xt_h = np.ascontiguousarray(x_pad.T.reshape(NKT, 128, TOK))
        in_maps.append(
            {
                "xt": xt_h,
                "wqk": wqk_h,
                "wo": wo_h,
                "qkvb": qkvb_h,
                "qkvb2": qkvb2_h,
                "outb": outb_h,
                "esink": esink_h,
                "costab": tabs[half][0],
                "sintab": tabs[half][1],
                "mask": masks[half],
                "ones1": ones_h,
                "ident": ident_h,
                "vones": vones_h,
                "pairsel": pair_h,
            }
        )
    return in_maps


def kernel(x, qkv_w, qkv_b, out_w, out_b, sinks, _trace=False, _tmpdir=None):
    nc = _get_nc()
    in_maps = _host_inputs(x, qkv_w, qkv_b, out_w, out_b, sinks)
    kwargs = {}
    if _trace:
        kwargs = dict(trace=True, tmpdir=_tmpdir)
    res = run_bass_kernel_spmd(nc, in_maps, core_ids=list(range(8)), **kwargs)
    out = np.empty((B, T, D), np.float32)
    for core in range(8):
        b, half = core // 2, core % 2
        out[b, half * TQ : half * TQ + TQ] = res.results[core]["y"]
    if _trace:
        kernel._last_results = res
    return out
